# revision 5
# baseline (speedup 1.0000x reference)
"""Trainium2 Bass kernel for nn_CausalPredictor_46462956208724.

Math: the reference computes
    wy = xm @ Wy_w.T + Wy_b            [L, 1]
    wz = dic_z @ Wz_w.T + Wz_b         [1, 1]
    attention = softmax(wy @ wz.T, axis=1)   # axis of size 1 -> exactly 1.0
    z = (attention * prior) @ dic_z    [L, C]
Softmax over a size-1 axis is exactly 1.0 in fp32 (exp(0)/exp(0) = 1/1),
so z[l, :] = prior[0] * dic[1, 0, :] for every row l, independent of xm.
The output is a broadcast of one scaled 1024-float row to 131072 rows -
a pure HBM-write problem (512 MB of output).

Sharding: pure data parallel over rows. 8 cores x 16384 rows each; the
tiny scaled row (prescaled by prior on the host - 1024 f32 multiplies,
same single-rounding as the device would do) is replicated to every
core as a 16x-tiled 64 KB buffer so the kernel needs no arithmetic.

Per-core schedule (measured on HW; all 16 SDMA engines run at the
~27.1 GB/s/engine SBUF-AXI port line rate once streaming, so the only
optimizable parts are the head before line-rate streaming and the
descriptor sizes):
  1. D2D prelude: rows 0:3 of each partition group stored DRAM->DRAM
     straight from row16 (12 KB descs). D2D is slow (~21 GB/s/engine)
     but has no SBUF dependency, so it keeps the engines busy during
     the window (~8.7-15 us) when nothing else is storable.
  2. Concurrently, DMA the 4 KB row into big[:, 0:C] of a [128, 16K]
     SBUF tile (stride-0 DRAM-side partition broadcast), then DVE
     log-doubling chain C->2C->4C->8C->16C; each completed doubling
     unlocks a bigger-descriptor wave: A rows 3:8 (4 KB descs, sync),
     B1 rows 8:12 (8 KB, scalar), B2 rows 12:16 (16 KB, scalar),
     C1 rows 16:48 (32 KB, sync), C2 rows 48:128 (64 KB desc = the
     framework MAX_DMA_LAST_DIM cap, sync).
  3. Output rows are partition-contiguous (partition p <-> rows
     [p*128,(p+1)*128)) so descriptor runs are contiguous on both
     sides.
Measured clean-machine time: ~169.7-170.4 us/core: ~8.7 us NEFF boot
preamble (fixed floor: first engine packet at ~8.7 in even a 1-DMA
NEFF), ~157 us of 100%-occupied engine streaming (64 MB at the fabric
ceiling + small-desc/D2D drag), ~2.3-4.4 us completion-receipt and
teardown-barrier tail. Runs on this shared machine frequently show a
uniform ~30% HBM slowdown from external interference (all engines'
64KB packets stretch 2420 -> 3400 ns); min over reps is the stable
statistic.

Rejected alternatives (measured): DRAM->DRAM stride-0 stores run at
~9 GB/s/engine (464 us full-D2D); descriptors >64 KB are rejected by
bass (MAX_DMA_LAST_DIM); splitting the bulk across both HWDGE rings
does not add bandwidth (the 16 SDMA engines/ports are the bottleneck,
shared by all queues).
"""

import sys

for _p in (
    "/root/.axon_site",
    "/root/.axon_site/_ro/trn_rl_repo",
    "/root/.axon_site/_ro/pypackages",
    "/opt/trn_rl_repo",
):
    if _p not in sys.path:
        sys.path.append(_p)

import numpy as np

L = 131072
C = 1024
N_CORES = 8
SHARD = L // N_CORES          # 16384 rows per core
P = 128                       # SBUF partitions

_CACHE = {}


def _build_bass():
    import concourse.bacc as bacc
    import concourse.tile as tile
    from concourse import mybir

    f32 = mybir.dt.float32
    # Bacc (not raw Bass): its compile() pipeline splits multi-sem waits
    # into event semaphores - TRN2 allows at most 1 wait per instruction,
    # and walrus rejects the raw IR with "Too many sync wait commands".
    nc = bacc.Bacc(None)
    row16_in = nc.declare_dram_parameter("row16", [1, 16 * C], f32, isOutput=False)
    out = nc.declare_dram_parameter("out", [SHARD, C], f32, isOutput=True)

    with tile.TileContext(nc) as tc:
        with tc.tile_pool(name="pool", bufs=1) as pool:
            out_pc = out[:].rearrange("(p r) c -> p r c", p=P)  # [128,128,1024]
            big = pool.tile([P, 16 * C], f32)
            # D2D prelude on scalar: rows 0:3 straight from DRAM row16
            # (12KB descs). DRAM->DRAM runs at only ~21 GB/s/engine, but it
            # needs no SBUF data, so it fills the engines' otherwise-idle
            # window before the load-completion semaphore fires. Kept to 3
            # rows (6 descs/engine): D2D descs run ~13% slower on the
            # high-index engines, and a larger dose skews their finish
            # times (the fully-packed schedule carries start/rate skew
            # straight to the last byte).
            nc.scalar.dma_start(
                out=out_pc[:, 0:3, :],
                in_=row16_in[:, 0 : 3 * C].partition_broadcast(P),
            )
            nc.sync.dma_start(
                out=big[:, 0:C], in_=row16_in[:, 0:C].partition_broadcast(P)
            )
            # A: rows 3:8 from big[0:C] on sync (2.5 MB, 4KB desc)
            nc.sync.dma_start(
                out=out_pc[:, 3:8, :],
                in_=big[:, 0:C].unsqueeze(1).broadcast_to([P, 5, C]),
            )
            nc.vector.tensor_copy(big[:, C : 2 * C], big[:, 0:C])
            nc.vector.tensor_copy(big[:, 2 * C : 4 * C], big[:, 0 : 2 * C])
            # B1: rows 8:12 on scalar (2 MB, 8KB desc)
            nc.scalar.dma_start(
                out=out_pc[:, 8:12, :],
                in_=big[:, 0 : 2 * C].unsqueeze(1).broadcast_to([P, 2, 2 * C]),
            )
            # B2: rows 12:16 on scalar (2 MB, 16KB desc)
            nc.scalar.dma_start(out=out_pc[:, 12:16, :], in_=big[:, 0 : 4 * C])
            nc.vector.tensor_copy(big[:, 4 * C : 8 * C], big[:, 0 : 4 * C])
            # C1: rows 16:48 on sync (16 MB, 32KB desc), starts after cp4
            nc.sync.dma_start(
                out=out_pc[:, 16:48, :],
                in_=big[:, 0 : 8 * C].unsqueeze(1).broadcast_to([P, 4, 8 * C]),
            )
            nc.vector.tensor_copy(big[:, 8 * C : 16 * C], big[:, 0 : 8 * C])
            # C2: rows 48:128 on sync (40 MB, 64KB desc), after cp5
            nc.sync.dma_start(
                out=out_pc[:, 48:128, :],
                in_=big[:, 0 : 16 * C].unsqueeze(1).broadcast_to([P, 5, 16 * C]),
            )
    nc.compile()
    return nc


def _get_nc():
    if "nc" not in _CACHE:
        _CACHE["nc"] = _build_bass()
    return _CACHE["nc"]


def _make_row16(dic, prior):
    row = np.asarray(dic, dtype=np.float32)[1].reshape(1, C)
    pr = np.asarray(prior, dtype=np.float32).reshape(())
    scaled = (row * pr).astype(np.float32)
    return np.ascontiguousarray(np.tile(scaled, (1, 16)))


def kernel(x, xm, Wy_w, Wy_b, Wz_w, Wz_b, dic, prior, **_unused):
    from concourse.bass_utils import run_bass_kernel_spmd

    nc = _get_nc()
    row16 = _make_row16(dic, prior)
    in_maps = [{"row16": row16} for _ in range(N_CORES)]
    last_err = None
    for _attempt in range(3):
        try:
            res = run_bass_kernel_spmd(nc, in_maps, list(range(N_CORES)))
            break
        except Exception as e:  # rare transient NRT device faults
            last_err = e
    else:
        raise last_err
    shards = [res.results[i]["out"] for i in range(N_CORES)]
    full = np.concatenate(shards, axis=0).reshape(L, 1, C)
    return full
